# revision 29
# baseline (speedup 1.0000x reference)
"""Trainium2 Bass kernel for additive (Bahdanau) attention.

Reference computation (per batch b):
    qp = queries @ Wq                    # (Tq, H)
    kp = keys @ Wk                       # (Tk, H)
    scores[q,k] = sum_h wv[h] * tanh(qp[q,h] + kp[k,h])
    attn = softmax(scores masked to k < valid_lens[b])
    out = attn @ values                  # (Tq, D)

Shapes: B=8, Tq=128, Tk=512, D=256, H=256 (fp32).

Strategy: separable harmonic expansion of tanh + key-chunk sharding.

The baseline's cost was the (q,k,h) tanh feature tensor on ScalarE
(1 elem/cycle/lane, ~58us on the critical core). This kernel removes
that tensor entirely: tanh(a+b) is approximated by an odd-harmonic sine
series  tanh(s) ~= sum_m c_m sin(m*om0*s), m in {1,3,..,13},  and each
sin(m*om0*(a+b)) factors exactly as
    sin(m*om0*a)cos(m*om0*b) + cos(m*om0*a)sin(m*om0*b),
so the whole score tensor becomes a TensorE matmul with contraction
(m, sin/cos, h) of size 2*7*H. Fit (Gaussian-weighted over the realized
s-distribution, |s|<=8.7): wrms ~1.1e-3, max err @|s|<=8.5 ~9e-3 -- below
the bf16 noise floor of the baseline.

Per chunk of 128 keys, on-core:
  - qp/kp projections on TensorE (bf16 inputs, fp32 PSUM);
  - fundamentals sin(om0*p), cos(om0*p) on ScalarE straight from PSUM
    (om0=0.28 keeps |angle| <= pi/2 for |p|<=5.6, within the Sin table's
    +-pi valid range even with the +pi/2 cos bias);
  - higher odd harmonics via the stride-2 Chebyshev/angle recurrence
    X_{m+2} = 2cos(2th) * X_m - X_{m-2} on DVE in bf16 (2x packed mode),
    both sides and sin/cos batched per instruction;
  - A-side scaled once by c_m * wv_h (precomputed, replicated constant);
  - 28 accumulating TensorE matmuls -> transposed score tile scT[k,q];
  - softmax via exp(scT + bias) with the global bound M = sum|wv|+1
    (partials combine across chunks by plain summation; Exp instructions
    for all chunks are grouped after all Sin instructions so the
    activation-table switch happens exactly once);
  - attn-partial @ [values | 1] on TensorE accumulates the denominator.
Host sums per-chunk [128, 257] partials per batch and divides.

Valid-length chunk planning as the baseline: only chunks with k <
valid_lens[b] are computed; chunks are padded to a uniform U per core.
"""

import math
import numpy as np
import ml_dtypes
from contextlib import ExitStack

import concourse.bass as bass
import concourse.tile as tile
from concourse import bacc, mybir
from concourse import bass_utils

B, Tq, Tk, D, H = 8, 128, 512, 256, 256
N_CORES = 8
KC = 128          # keys per chunk
F32 = mybir.dt.float32
BF16 = mybir.dt.bfloat16
NEG_BIG = -1.0e9

# odd-harmonic sine fit of tanh: tanh(s) ~= sum_j CM[j] sin((2j+1)*OM0*s)
OM0 = 0.296
CM = [1.23023, 0.31007, 0.12172, 0.03782, 0.02813]
KH = len(CM)

SIN = mybir.ActivationFunctionType.Sin
EXP = mybir.ActivationFunctionType.Exp
MULT = mybir.AluOpType.mult
ADD = mybir.AluOpType.add
SUB = mybir.AluOpType.subtract


def _bcast(ap_slice, axis_idx, count):
    """Insert a step-0 (broadcast) dim into an AP (axis_idx includes the
    partition dim at index 0)."""
    ap = list(ap_slice.ap)
    ap.insert(axis_idx, [0, count])
    return bass.AP(tensor=ap_slice.tensor, offset=ap_slice.offset, ap=ap)


def _flat(ap_slice, keep=0):
    """Merge the trailing free dims of a contiguous slice into one long
    row (DVE pays a per-row overhead, so fewer/longer rows are faster).
    `keep` leading free dims are preserved (e.g. a step-0 broadcast dim)."""
    ap = list(ap_slice.ap)
    head, tail = ap[: 1 + keep], ap[1 + keep :]
    n = 1
    for _, ct in tail:
        n *= ct
    return bass.AP(tensor=ap_slice.tensor, offset=ap_slice.offset,
                   ap=head + [[1, n]])


def _emit(nc, tc, ins, out_dram, U):
    with ExitStack() as ctx:
        const = ctx.enter_context(tc.tile_pool(name="const", bufs=1))
        chunk_in = ctx.enter_context(tc.tile_pool(name="chunk_in", bufs=2))
        feat = ctx.enter_context(tc.tile_pool(name="feat", bufs=2))
        scal = ctx.enter_context(tc.tile_pool(name="scal", bufs=2))
        pt_pool = ctx.enter_context(tc.tile_pool(name="pt", bufs=2))
        out_pool = ctx.enter_context(tc.tile_pool(name="outs", bufs=2))
        proj_ps = ctx.enter_context(tc.tile_pool(name="proj_ps", bufs=2, space="PSUM"))
        sc_ps_pool = ctx.enter_context(tc.tile_pool(name="sc_ps", bufs=1, space="PSUM"))
        av_ps_pool = ctx.enter_context(tc.tile_pool(name="av_ps", bufs=2, space="PSUM"))

        # pi/2 bias for the cos-via-sin fundamentals
        halfpi = const.tile([128, 1], F32)
        nc.vector.memset(halfpi, float(np.pi / 2))

        wk_sb = const.tile([128, 2, H], BF16)
        wq_sb = const.tile([128, 2, H], BF16)
        # wvc shipped tiny ([128, KH, half]) and replicated on-device over
        # (sc, q) by two ScalarE Copies -- saves ~650KB of DMA traffic.
        wvc_in = const.tile([128, KH, 2], BF16)
        wvc_sb = const.tile([128, KH, 2, 2, Tq], BF16)

        # ---- inputs on three DMA queues, balanced so the projection inputs
        # all land ~together: the combined all-chunk kT/qT transfers (packed
        # [128, U, 2, T] on host) ride with one weight half each; the other
        # weight halves go via the scalar queue.
        queues = [nc.sync, nc.gpsimd, nc.scalar]
        kT_sb = chunk_in.tile([128, U, 2, KC], BF16, tag="kT")
        qT_sb = chunk_in.tile([128, U, 2, Tq], BF16, tag="qT")
        nc.scalar.dma_start(out=wvc_in, in_=ins["wvc"])
        nc.sync.dma_start(out=wk_sb[:, 0], in_=ins["wk"][:, 0])
        nc.gpsimd.dma_start(out=wq_sb[:, 0], in_=ins["wq"][:, 0])
        nc.scalar.dma_start(out=wk_sb[:, 1], in_=ins["wk"][:, 1])
        nc.scalar.dma_start(out=wq_sb[:, 1], in_=ins["wq"][:, 1])
        nc.sync.dma_start(out=kT_sb, in_=ins["kT_u"])
        nc.gpsimd.dma_start(out=qT_sb, in_=ins["qT_u"])

        # Dummy activation: pulls the Sin table load off the critical path
        # (runs while the DMAs above are in flight).
        warm_sb = const.tile([1, 1], F32)
        nc.vector.memset(warm_sb, 0.0)
        nc.scalar.activation(warm_sb, warm_sb, SIN)
        # PE pipeline warmup.
        warm_w = const.tile([1, 2], BF16)
        nc.vector.memset(warm_w, 0.0)
        wp = av_ps_pool.tile([1, 1], F32, tag="avo")
        nc.tensor.matmul(wp, warm_w[:, 0:1], warm_w[:, 1:2], start=True, stop=True)

        # ---- phase A: all-chunk projections + two big fundamentals ----
        # X layout: [128p(h), KH, sc(sin=0,cos=1), U, side(q=0,k=1), half, T]
        X = feat.tile([128, KH, 2, U, 2, 2, Tq], BF16, tag="X")
        # one PSUM region [U, side, half, T] matching X's fundamental slice
        pk_ps = proj_ps.tile([128, U, 2, 2, Tq], F32, tag="pk")
        for u in range(U):
            for half in range(2):
                hs = slice(half * 128, (half + 1) * 128)
                for dc in range(2):
                    nc.tensor.matmul(
                        pk_ps[:, u, 1, half, :], wk_sb[:, dc, hs],
                        kT_sb[:, u, dc, :], start=(dc == 0), stop=(dc == 1))
            for half in range(2):
                hs = slice(half * 128, (half + 1) * 128)
                for dc in range(2):
                    nc.tensor.matmul(
                        pk_ps[:, u, 0, half, :], wq_sb[:, dc, hs],
                        qT_sb[:, u, dc, :], start=(dc == 0), stop=(dc == 1))
        # fundamentals: sin first (it alone gates the ladder's t2/dpm chain)
        nc.scalar.activation(X[:, 0, 0], pk_ps, SIN, scale=OM0)
        nc.scalar.activation(X[:, 0, 1], pk_ps, SIN, bias=halfpi, scale=OM0)
        # Replicate wvc over (sc, q) on ScalarE -- after the fundamentals so
        # it never delays them; only needed once the ladder reaches wvc01.
        # in: [p][2,KH][1,2][0,Tq] (m, half packed, q bcast); out per sc.
        wvc_flat = bass.AP(tensor=wvc_in.tensor, offset=wvc_in.offset,
                           ap=[list(wvc_in.ap[0]), [2, KH], [1, 2], [0, Tq]])
        for sc in range(2):
            o = wvc_sb[:, :, sc]
            o_ap = bass.AP(tensor=o.tensor, offset=o.offset,
                           ap=[list(o.ap[0]), [2 * 2 * Tq, KH], [Tq, 2], [1, Tq]])
            nc.scalar.activation(o_ap, wvc_flat,
                                 mybir.ActivationFunctionType.Copy)

        # ---- odd-harmonic ladder on DVE, all chunks batched in one op set --
        s1 = X[:, 0, 0]   # [128, U, side, half, T]
        t2 = scal.tile([128, U, 2, 2, Tq], BF16, tag="t2")
        nc.vector.tensor_tensor(out=t2, in0=s1, in1=s1, op=MULT)
        # dpm[0] = d+1 = 3-4s1^2 (pairs sin), dpm[1] = d-1 (pairs cos)
        dpm = scal.tile([128, 2, U, 2, 2, Tq], BF16, tag="dpm")
        dd = scal.tile([128, U, 2, 2, Tq], BF16, tag="dd")    # d = 2-4s1^2
        nc.vector.tensor_scalar(out=dpm[:, 0], in0=t2, scalar1=-4.0,
                                scalar2=3.0, op0=MULT, op1=ADD)
        nc.vector.tensor_scalar(out=dpm[:, 1], in0=t2, scalar1=-4.0,
                                scalar2=1.0, op0=MULT, op1=ADD)
        nc.vector.tensor_scalar(out=dd, in0=t2, scalar1=-4.0,
                                scalar2=2.0, op0=MULT, op1=ADD)
        Ap = feat.tile([128, KH, 2, U, 2, Tq], BF16, tag="Ap")

        def emit_wvc(mr):
            # A-side scale of levels [mr] by c_m * wv_h (bcast over U)
            nc.vector.tensor_tensor(
                out=Ap[:, mr], in0=X[:, mr, :, :, 0],
                in1=_bcast(wvc_sb[:, mr], 3, U), op=MULT)

        # m=3: X[1] = X[0] * dpm   (sc-paired multipliers)
        nc.vector.tensor_tensor(out=X[:, 1], in0=X[:, 0], in1=dpm, op=MULT)
        emit_wvc(slice(0, 2))
        # m>=5: X[lv] = d*X[lv-1] - X[lv-2]   (d bcast over sc); per-level
        # wvc lets the PE start that level's score matmuls immediately.
        for lv in range(2, KH):
            P = scal.tile([128, 2, U, 2, 2, Tq], BF16, tag="P")
            nc.vector.tensor_tensor(out=P, in0=X[:, lv - 1],
                                    in1=_bcast(dd, 1, 2), op=MULT)
            nc.vector.tensor_tensor(out=X[:, lv], in0=P, in1=X[:, lv - 2],
                                    op=SUB)
            emit_wvc(slice(lv, lv + 1))

        # ---- scores: scT[k,q] += B_chunk^T A_chunk over (m, sc, half) ----
        scts = [sc_ps_pool.tile([128, Tq], F32, tag=f"scT{u}", name=f"scT{u}")
                for u in range(U)]
        for m in range(KH):
            for pi, (scb, sca) in enumerate(((1, 0), (0, 1))):
                for u in range(U):
                    for half in range(2):
                        nc.tensor.matmul(
                            scts[u], X[:, m, scb, u, 1, half, :],
                            Ap[:, m, sca, u, half, :],
                            start=(m == 0 and pi == 0 and half == 0),
                            stop=(m == KH - 1 and pi == 1 and half == 1))

        # ---- phase B: all Exp instructions grouped (one table switch) ----
        for u in range(U):
            b_eng = queues[u % 2]  # sync / gpsimd (keep scalar free for ACT)
            v_sb = chunk_in.tile([128, D + 1], BF16, tag="v")
            b_eng.dma_start(out=v_sb, in_=ins["v_u"][u])
            mb_sb = chunk_in.tile([128, 1], F32, tag="mb")
            b_eng.dma_start(out=mb_sb, in_=ins["mb_u"][u])
            pT_sb = pt_pool.tile([128, Tq], BF16, tag="pT")
            nc.scalar.activation(pT_sb, scts[u], EXP, bias=mb_sb[:, 0:1], scale=1.0)
            av_ps = av_ps_pool.tile([Tq, D + 1], F32, tag="avo")
            nc.tensor.matmul(av_ps, pT_sb, v_sb, start=True, stop=True)
            out_sb = out_pool.tile([Tq, D + 1], F32, tag="out")
            nc.vector.tensor_copy(out_sb, av_ps)
            nc.sync.dma_start(out=out_dram[u], in_=out_sb)


def _build(U):
    nc = bacc.Bacc(
        "TRN2",
        target_bir_lowering=False,
        debug=False,
        enable_asserts=False,
        num_devices=N_CORES,
    )
    ins = {
        "wq": nc.dram_tensor("wq", [128, 2, H], BF16, kind="ExternalInput").ap(),
        "wk": nc.dram_tensor("wk", [128, 2, H], BF16, kind="ExternalInput").ap(),
        "wvc": nc.dram_tensor("wvc", [128, KH, 2], BF16, kind="ExternalInput").ap(),
        "qT_u": nc.dram_tensor("qT_u", [128, U, 2, Tq], BF16, kind="ExternalInput").ap(),
        "kT_u": nc.dram_tensor("kT_u", [128, U, 2, KC], BF16, kind="ExternalInput").ap(),
        "v_u": nc.dram_tensor("v_u", [U, KC, D + 1], BF16, kind="ExternalInput").ap(),
        "mb_u": nc.dram_tensor("mb_u", [U, KC, 1], F32, kind="ExternalInput").ap(),
    }
    out_dram = nc.dram_tensor("out_u", [U, Tq, D + 1], F32, kind="ExternalOutput").ap()
    with tile.TileContext(nc) as tc:
        _emit(nc, tc, ins, out_dram, U)
    nc.compile()
    return nc


_NC_CACHE = {}


def _get_nc(U):
    if U not in _NC_CACHE:
        _NC_CACHE[U] = _build(U)
    return _NC_CACHE[U]


def _plan_chunks(valid_lens):
    chunks = []
    for b in range(B):
        n = int(valid_lens[b])
        for kc in range(math.ceil(max(n, 0) / KC)):
            chunks.append((b, kc))
    U = max(1, math.ceil(len(chunks) / N_CORES))
    chunks += [None] * (N_CORES * U - len(chunks))
    return chunks, U


def run(queries, keys, values, valid_lens, Wq, Wk, wv, trace=False):
    """Run the SPMD kernel; returns (output, BassKernelResults)."""
    queries = np.asarray(queries, dtype=np.float32)
    keys = np.asarray(keys, dtype=np.float32)
    values = np.asarray(values, dtype=np.float32)
    valid_lens = np.asarray(valid_lens)

    def pmajor(a):
        # [d, ...] -> [p, c, ...] with d = c*128 + p, contiguous
        return np.ascontiguousarray(
            a.reshape(2, 128, *a.shape[1:]).swapaxes(0, 1)
        )

    Wq_p = pmajor(np.asarray(Wq, dtype=np.float32).astype(ml_dtypes.bfloat16))
    Wk_p = pmajor(np.asarray(Wk, dtype=np.float32).astype(ml_dtypes.bfloat16))
    wv_bf = np.asarray(wv, dtype=np.float32).astype(ml_dtypes.bfloat16)
    # scores are bounded by ~sum|wv|; M makes exp(s-M) overflow-safe without
    # a row max, so partial softmax sums combine by addition.
    M = float(np.abs(wv_bf.astype(np.float32)).sum()) + 1.0

    # wvc[p, m, half] = CM[m] * wv[half*128 + p] (device replicates over sc, q)
    wv_ph = wv_bf.astype(np.float32).reshape(2, 128).T        # [128p, 2half]
    wvc = np.ascontiguousarray(
        np.asarray(CM, np.float32)[None, :, None] * wv_ph[:, None, :]
    ).astype(ml_dtypes.bfloat16)

    chunks, U = _plan_chunks(valid_lens)
    nc = _get_nc(U)

    # [B, D, T] transposed inputs, packed partition-major per batch
    qT = np.stack([pmajor(queries[b].T.astype(ml_dtypes.bfloat16)) for b in range(B)])
    kT = np.stack([pmajor(keys[b].T.astype(ml_dtypes.bfloat16)) for b in range(B)])
    ones = np.ones((KC, 1), dtype=np.float32)
    arange = np.arange(KC)

    in_maps = []
    for c in range(N_CORES):
        qT_u = np.zeros((128, U, 2, Tq), ml_dtypes.bfloat16)
        kT_u = np.zeros((128, U, 2, KC), ml_dtypes.bfloat16)
        v_u = np.zeros((U, KC, D + 1), ml_dtypes.bfloat16)
        mb_u = np.full((U, KC, 1), NEG_BIG - M, np.float32)
        for u in range(U):
            ch = chunks[c * U + u]
            if ch is None:
                continue
            b, kc = ch
            k0 = kc * KC
            qT_u[:, u] = qT[b]
            kT_u[:, u] = kT[b][:, :, k0 : k0 + KC]
            v_u[u] = np.concatenate([values[b][k0 : k0 + KC], ones], axis=1).astype(
                ml_dtypes.bfloat16
            )
            mb_u[u, :, 0] = (
                np.where(k0 + arange < int(valid_lens[b]), 0.0, NEG_BIG) - M
            ).astype(np.float32)
        in_maps.append(
            {
                "wq": Wq_p,
                "wk": Wk_p,
                "wvc": wvc,
                "qT_u": qT_u,
                "kT_u": kT_u,
                "v_u": v_u,
                "mb_u": mb_u,
            }
        )

    res = bass_utils.run_bass_kernel_spmd(
        nc, in_maps, core_ids=list(range(N_CORES)), trace=trace
    )

    acc = np.zeros((B, Tq, D + 1), np.float64)
    for c in range(N_CORES):
        part = res.results[c]["out_u"]  # [U, Tq, D+1]
        for u in range(U):
            ch = chunks[c * U + u]
            if ch is None:
                continue
            acc[ch[0]] += part[u]
    out = np.zeros((B, Tq, D), np.float32)
    for b in range(B):
        if int(valid_lens[b]) > 0:
            out[b] = (acc[b, :, :D] / acc[b, :, D : D + 1]).astype(np.float32)
    return out, res


def kernel(queries, keys, values, valid_lens, Wq, Wk, wv):
    out, _ = run(queries, keys, values, valid_lens, Wq, Wk, wv, trace=False)
    return out


# revision 30
# speedup vs baseline: 1.1670x; 1.1670x over previous
"""Trainium2 Bass kernel for additive (Bahdanau) attention.

Reference computation (per batch b):
    qp = queries @ Wq                    # (Tq, H)
    kp = keys @ Wk                       # (Tk, H)
    scores[q,k] = sum_h wv[h] * tanh(qp[q,h] + kp[k,h])
    attn = softmax(scores masked to k < valid_lens[b])
    out = attn @ values                  # (Tq, D)

Shapes: B=8, Tq=128, Tk=512, D=256, H=256 (fp32).

Strategy: separable harmonic expansion of tanh + key-chunk sharding.

The baseline's cost was the (q,k,h) tanh feature tensor on ScalarE
(1 elem/cycle/lane, ~58us on the critical core). This kernel removes
that tensor entirely: tanh(a+b) is approximated by an odd-harmonic sine
series  tanh(s) ~= sum_m c_m sin(m*om0*s), m in {1,3,..,13},  and each
sin(m*om0*(a+b)) factors exactly as
    sin(m*om0*a)cos(m*om0*b) + cos(m*om0*a)sin(m*om0*b),
so the whole score tensor becomes a TensorE matmul with contraction
(m, sin/cos, h) of size 2*7*H. Fit (Gaussian-weighted over the realized
s-distribution, |s|<=8.7): wrms ~1.1e-3, max err @|s|<=8.5 ~9e-3 -- below
the bf16 noise floor of the baseline.

Per chunk of 128 keys, on-core:
  - qp/kp projections on TensorE (bf16 inputs, fp32 PSUM);
  - fundamentals sin(om0*p), cos(om0*p) on ScalarE straight from PSUM
    (om0=0.28 keeps |angle| <= pi/2 for |p|<=5.6, within the Sin table's
    +-pi valid range even with the +pi/2 cos bias);
  - higher odd harmonics via the stride-2 Chebyshev/angle recurrence
    X_{m+2} = 2cos(2th) * X_m - X_{m-2} on DVE in bf16 (2x packed mode),
    both sides and sin/cos batched per instruction;
  - A-side scaled once by c_m * wv_h (precomputed, replicated constant);
  - 28 accumulating TensorE matmuls -> transposed score tile scT[k,q];
  - softmax via exp(scT + bias) with the global bound M = sum|wv|+1
    (partials combine across chunks by plain summation; Exp instructions
    for all chunks are grouped after all Sin instructions so the
    activation-table switch happens exactly once);
  - attn-partial @ [values | 1] on TensorE accumulates the denominator.
Host sums per-chunk [128, 257] partials per batch and divides.

Valid-length chunk planning as the baseline: only chunks with k <
valid_lens[b] are computed; chunks are padded to a uniform U per core.
"""

import math
import numpy as np
import ml_dtypes
from contextlib import ExitStack

import concourse.bass as bass
import concourse.tile as tile
from concourse import bacc, mybir
from concourse import bass_utils

B, Tq, Tk, D, H = 8, 128, 512, 256, 256
N_CORES = 8
KC = 128          # keys per chunk
F32 = mybir.dt.float32
BF16 = mybir.dt.bfloat16
NEG_BIG = -1.0e9

# odd-harmonic sine fit of tanh: tanh(s) ~= sum_j CM[j] sin((2j+1)*OM0*s)
OM0 = 0.296
CM = [1.23023, 0.31007, 0.12172, 0.03782, 0.02813]
KH = len(CM)

SIN = mybir.ActivationFunctionType.Sin
EXP = mybir.ActivationFunctionType.Exp
MULT = mybir.AluOpType.mult
ADD = mybir.AluOpType.add
SUB = mybir.AluOpType.subtract


def _bcast(ap_slice, axis_idx, count):
    """Insert a step-0 (broadcast) dim into an AP (axis_idx includes the
    partition dim at index 0)."""
    ap = list(ap_slice.ap)
    ap.insert(axis_idx, [0, count])
    return bass.AP(tensor=ap_slice.tensor, offset=ap_slice.offset, ap=ap)


def _flat(ap_slice, keep=0):
    """Merge the trailing free dims of a contiguous slice into one long
    row (DVE pays a per-row overhead, so fewer/longer rows are faster).
    `keep` leading free dims are preserved (e.g. a step-0 broadcast dim)."""
    ap = list(ap_slice.ap)
    head, tail = ap[: 1 + keep], ap[1 + keep :]
    n = 1
    for _, ct in tail:
        n *= ct
    return bass.AP(tensor=ap_slice.tensor, offset=ap_slice.offset,
                   ap=head + [[1, n]])


def _emit(nc, tc, ins, out_dram, U):
    with ExitStack() as ctx:
        const = ctx.enter_context(tc.tile_pool(name="const", bufs=1))
        chunk_in = ctx.enter_context(tc.tile_pool(name="chunk_in", bufs=2))
        feat = ctx.enter_context(tc.tile_pool(name="feat", bufs=2))
        scal = ctx.enter_context(tc.tile_pool(name="scal", bufs=2))
        pt_pool = ctx.enter_context(tc.tile_pool(name="pt", bufs=2))
        out_pool = ctx.enter_context(tc.tile_pool(name="outs", bufs=2))
        proj_ps = ctx.enter_context(tc.tile_pool(name="proj_ps", bufs=2, space="PSUM"))
        sc_ps_pool = ctx.enter_context(tc.tile_pool(name="sc_ps", bufs=1, space="PSUM"))
        av_ps_pool = ctx.enter_context(tc.tile_pool(name="av_ps", bufs=2, space="PSUM"))

        # pi/2 bias for the cos-via-sin fundamentals
        halfpi = const.tile([128, 1], F32)
        nc.vector.memset(halfpi, float(np.pi / 2))

        wk_sb = const.tile([128, 2, H], BF16)
        wq_sb = const.tile([128, 2, H], BF16)
        # wvc shipped tiny ([128, KH, half]) and replicated on-device over
        # (sc, q) by two ScalarE Copies -- saves ~650KB of DMA traffic.
        wvc_in = const.tile([128, KH, 2], BF16)
        wvc_sb = const.tile([128, KH, 2, 2, Tq], BF16)

        # ---- inputs on three DMA queues, balanced so the projection inputs
        # all land ~together: the combined all-chunk kT/qT transfers (packed
        # [128, U, 2, T] on host) ride with one weight half each; the other
        # weight halves go via the scalar queue.
        queues = [nc.sync, nc.gpsimd, nc.scalar]
        kT_sb = chunk_in.tile([128, U, 2, KC], BF16, tag="kT")
        qT_sb = chunk_in.tile([128, U, 2, Tq], BF16, tag="qT")
        nc.scalar.dma_start(out=wvc_in, in_=ins["wvc"])
        nc.sync.dma_start(out=wk_sb, in_=ins["wk"])
        nc.gpsimd.dma_start(out=wq_sb, in_=ins["wq"])
        nc.sync.dma_start(out=kT_sb, in_=ins["kT_u"])
        nc.gpsimd.dma_start(out=qT_sb, in_=ins["qT_u"])

        # Dummy activation: pulls the Sin table load off the critical path
        # (runs while the DMAs above are in flight).
        warm_sb = const.tile([1, 1], F32)
        nc.vector.memset(warm_sb, 0.0)
        nc.scalar.activation(warm_sb, warm_sb, SIN)
        # PE pipeline warmup.
        warm_w = const.tile([1, 2], BF16)
        nc.vector.memset(warm_w, 0.0)
        wp = av_ps_pool.tile([1, 1], F32, tag="avo")
        nc.tensor.matmul(wp, warm_w[:, 0:1], warm_w[:, 1:2], start=True, stop=True)

        # ---- phase A: all-chunk projections + two big fundamentals ----
        # X layout: [128p(h), KH, sc(sin=0,cos=1), U, side(q=0,k=1), half, T]
        X = feat.tile([128, KH, 2, U, 2, 2, Tq], BF16, tag="X")
        # one PSUM region [U, side, half, T] matching X's fundamental slice
        pk_ps = proj_ps.tile([128, U, 2, 2, Tq], F32, tag="pk")
        for u in range(U):
            for half in range(2):
                hs = slice(half * 128, (half + 1) * 128)
                for dc in range(2):
                    nc.tensor.matmul(
                        pk_ps[:, u, 1, half, :], wk_sb[:, dc, hs],
                        kT_sb[:, u, dc, :], start=(dc == 0), stop=(dc == 1))
            for half in range(2):
                hs = slice(half * 128, (half + 1) * 128)
                for dc in range(2):
                    nc.tensor.matmul(
                        pk_ps[:, u, 0, half, :], wq_sb[:, dc, hs],
                        qT_sb[:, u, dc, :], start=(dc == 0), stop=(dc == 1))
        # fundamentals: sin first (it alone gates the ladder's t2/dpm chain)
        nc.scalar.activation(X[:, 0, 0], pk_ps, SIN, scale=OM0)
        nc.scalar.activation(X[:, 0, 1], pk_ps, SIN, bias=halfpi, scale=OM0)
        # Replicate wvc over (sc, q) on ScalarE -- after the fundamentals so
        # it never delays them; only needed once the ladder reaches wvc01.
        # in: [p][2,KH][1,2][0,Tq] (m, half packed, q bcast); out per sc.
        wvc_flat = bass.AP(tensor=wvc_in.tensor, offset=wvc_in.offset,
                           ap=[list(wvc_in.ap[0]), [2, KH], [1, 2], [0, Tq]])
        for sc in range(2):
            o = wvc_sb[:, :, sc]
            o_ap = bass.AP(tensor=o.tensor, offset=o.offset,
                           ap=[list(o.ap[0]), [2 * 2 * Tq, KH], [Tq, 2], [1, Tq]])
            nc.scalar.activation(o_ap, wvc_flat,
                                 mybir.ActivationFunctionType.Copy)

        # ---- odd-harmonic ladder on DVE, all chunks batched in one op set --
        s1 = X[:, 0, 0]   # [128, U, side, half, T]
        t2 = scal.tile([128, U, 2, 2, Tq], BF16, tag="t2")
        nc.vector.tensor_tensor(out=t2, in0=s1, in1=s1, op=MULT)
        # dpm[0] = d+1 = 3-4s1^2 (pairs sin), dpm[1] = d-1 (pairs cos)
        dpm = scal.tile([128, 2, U, 2, 2, Tq], BF16, tag="dpm")
        dd = scal.tile([128, U, 2, 2, Tq], BF16, tag="dd")    # d = 2-4s1^2
        nc.vector.tensor_scalar(out=dpm[:, 0], in0=t2, scalar1=-4.0,
                                scalar2=3.0, op0=MULT, op1=ADD)
        nc.vector.tensor_scalar(out=dpm[:, 1], in0=t2, scalar1=-4.0,
                                scalar2=1.0, op0=MULT, op1=ADD)
        nc.vector.tensor_scalar(out=dd, in0=t2, scalar1=-4.0,
                                scalar2=2.0, op0=MULT, op1=ADD)
        Ap = feat.tile([128, KH, 2, U, 2, Tq], BF16, tag="Ap")

        def emit_wvc(mr):
            # A-side scale of levels [mr] by c_m * wv_h (bcast over U)
            nc.vector.tensor_tensor(
                out=Ap[:, mr], in0=X[:, mr, :, :, 0],
                in1=_bcast(wvc_sb[:, mr], 3, U), op=MULT)

        # m=3: X[1] = X[0] * dpm   (sc-paired multipliers)
        nc.vector.tensor_tensor(out=X[:, 1], in0=X[:, 0], in1=dpm, op=MULT)
        emit_wvc(slice(0, 2))
        # m>=5: X[lv] = d*X[lv-1] - X[lv-2]   (d bcast over sc); per-level
        # wvc lets the PE start that level's score matmuls immediately.
        for lv in range(2, KH):
            P = scal.tile([128, 2, U, 2, 2, Tq], BF16, tag="P")
            nc.vector.tensor_tensor(out=P, in0=X[:, lv - 1],
                                    in1=_bcast(dd, 1, 2), op=MULT)
            nc.vector.tensor_tensor(out=X[:, lv], in0=P, in1=X[:, lv - 2],
                                    op=SUB)
            emit_wvc(slice(lv, lv + 1))

        # ---- scores: scT[k,q] += B_chunk^T A_chunk over (m, sc, half) ----
        scts = [sc_ps_pool.tile([128, Tq], F32, tag=f"scT{u}", name=f"scT{u}")
                for u in range(U)]
        for m in range(KH):
            for pi, (scb, sca) in enumerate(((1, 0), (0, 1))):
                for u in range(U):
                    for half in range(2):
                        nc.tensor.matmul(
                            scts[u], X[:, m, scb, u, 1, half, :],
                            Ap[:, m, sca, u, half, :],
                            start=(m == 0 and pi == 0 and half == 0),
                            stop=(m == KH - 1 and pi == 1 and half == 1))

        # ---- phase B: all Exp instructions grouped (one table switch) ----
        for u in range(U):
            b_eng = queues[u % 2]  # sync / gpsimd (keep scalar free for ACT)
            v_sb = chunk_in.tile([128, D + 1], BF16, tag="v")
            b_eng.dma_start(out=v_sb, in_=ins["v_u"][u])
            mb_sb = chunk_in.tile([128, 1], F32, tag="mb")
            b_eng.dma_start(out=mb_sb, in_=ins["mb_u"][u])
            pT_sb = pt_pool.tile([128, Tq], BF16, tag="pT")
            nc.scalar.activation(pT_sb, scts[u], EXP, bias=mb_sb[:, 0:1], scale=1.0)
            av_ps = av_ps_pool.tile([Tq, D + 1], F32, tag="avo")
            nc.tensor.matmul(av_ps, pT_sb, v_sb, start=True, stop=True)
            out_sb = out_pool.tile([Tq, D + 1], F32, tag="out")
            nc.vector.tensor_copy(out_sb, av_ps)
            nc.sync.dma_start(out=out_dram[u], in_=out_sb)


def _build(U):
    nc = bacc.Bacc(
        "TRN2",
        target_bir_lowering=False,
        debug=False,
        enable_asserts=False,
        num_devices=N_CORES,
    )
    ins = {
        "wq": nc.dram_tensor("wq", [128, 2, H], BF16, kind="ExternalInput").ap(),
        "wk": nc.dram_tensor("wk", [128, 2, H], BF16, kind="ExternalInput").ap(),
        "wvc": nc.dram_tensor("wvc", [128, KH, 2], BF16, kind="ExternalInput").ap(),
        "qT_u": nc.dram_tensor("qT_u", [128, U, 2, Tq], BF16, kind="ExternalInput").ap(),
        "kT_u": nc.dram_tensor("kT_u", [128, U, 2, KC], BF16, kind="ExternalInput").ap(),
        "v_u": nc.dram_tensor("v_u", [U, KC, D + 1], BF16, kind="ExternalInput").ap(),
        "mb_u": nc.dram_tensor("mb_u", [U, KC, 1], F32, kind="ExternalInput").ap(),
    }
    out_dram = nc.dram_tensor("out_u", [U, Tq, D + 1], F32, kind="ExternalOutput").ap()
    with tile.TileContext(nc) as tc:
        _emit(nc, tc, ins, out_dram, U)
    nc.compile()
    return nc


_NC_CACHE = {}


def _get_nc(U):
    if U not in _NC_CACHE:
        _NC_CACHE[U] = _build(U)
    return _NC_CACHE[U]


def _plan_chunks(valid_lens):
    chunks = []
    for b in range(B):
        n = int(valid_lens[b])
        for kc in range(math.ceil(max(n, 0) / KC)):
            chunks.append((b, kc))
    U = max(1, math.ceil(len(chunks) / N_CORES))
    chunks += [None] * (N_CORES * U - len(chunks))
    return chunks, U


def run(queries, keys, values, valid_lens, Wq, Wk, wv, trace=False):
    """Run the SPMD kernel; returns (output, BassKernelResults)."""
    queries = np.asarray(queries, dtype=np.float32)
    keys = np.asarray(keys, dtype=np.float32)
    values = np.asarray(values, dtype=np.float32)
    valid_lens = np.asarray(valid_lens)

    def pmajor(a):
        # [d, ...] -> [p, c, ...] with d = c*128 + p, contiguous
        return np.ascontiguousarray(
            a.reshape(2, 128, *a.shape[1:]).swapaxes(0, 1)
        )

    Wq_p = pmajor(np.asarray(Wq, dtype=np.float32).astype(ml_dtypes.bfloat16))
    Wk_p = pmajor(np.asarray(Wk, dtype=np.float32).astype(ml_dtypes.bfloat16))
    wv_bf = np.asarray(wv, dtype=np.float32).astype(ml_dtypes.bfloat16)
    # scores are bounded by ~sum|wv|; M makes exp(s-M) overflow-safe without
    # a row max, so partial softmax sums combine by addition.
    M = float(np.abs(wv_bf.astype(np.float32)).sum()) + 1.0

    # wvc[p, m, half] = CM[m] * wv[half*128 + p] (device replicates over sc, q)
    wv_ph = wv_bf.astype(np.float32).reshape(2, 128).T        # [128p, 2half]
    wvc = np.ascontiguousarray(
        np.asarray(CM, np.float32)[None, :, None] * wv_ph[:, None, :]
    ).astype(ml_dtypes.bfloat16)

    chunks, U = _plan_chunks(valid_lens)
    nc = _get_nc(U)

    # [B, D, T] transposed inputs, packed partition-major per batch
    qT = np.stack([pmajor(queries[b].T.astype(ml_dtypes.bfloat16)) for b in range(B)])
    kT = np.stack([pmajor(keys[b].T.astype(ml_dtypes.bfloat16)) for b in range(B)])
    ones = np.ones((KC, 1), dtype=np.float32)
    arange = np.arange(KC)

    in_maps = []
    for c in range(N_CORES):
        qT_u = np.zeros((128, U, 2, Tq), ml_dtypes.bfloat16)
        kT_u = np.zeros((128, U, 2, KC), ml_dtypes.bfloat16)
        v_u = np.zeros((U, KC, D + 1), ml_dtypes.bfloat16)
        mb_u = np.full((U, KC, 1), NEG_BIG - M, np.float32)
        for u in range(U):
            ch = chunks[c * U + u]
            if ch is None:
                continue
            b, kc = ch
            k0 = kc * KC
            qT_u[:, u] = qT[b]
            kT_u[:, u] = kT[b][:, :, k0 : k0 + KC]
            v_u[u] = np.concatenate([values[b][k0 : k0 + KC], ones], axis=1).astype(
                ml_dtypes.bfloat16
            )
            mb_u[u, :, 0] = (
                np.where(k0 + arange < int(valid_lens[b]), 0.0, NEG_BIG) - M
            ).astype(np.float32)
        in_maps.append(
            {
                "wq": Wq_p,
                "wk": Wk_p,
                "wvc": wvc,
                "qT_u": qT_u,
                "kT_u": kT_u,
                "v_u": v_u,
                "mb_u": mb_u,
            }
        )

    res = bass_utils.run_bass_kernel_spmd(
        nc, in_maps, core_ids=list(range(N_CORES)), trace=trace
    )

    acc = np.zeros((B, Tq, D + 1), np.float64)
    for c in range(N_CORES):
        part = res.results[c]["out_u"]  # [U, Tq, D+1]
        for u in range(U):
            ch = chunks[c * U + u]
            if ch is None:
                continue
            acc[ch[0]] += part[u]
    out = np.zeros((B, Tq, D), np.float32)
    for b in range(B):
        if int(valid_lens[b]) > 0:
            out[b] = (acc[b, :, :D] / acc[b, :, D : D + 1]).astype(np.float32)
    return out, res


def kernel(queries, keys, values, valid_lens, Wq, Wk, wv):
    out, _ = run(queries, keys, values, valid_lens, Wq, Wk, wv, trace=False)
    return out


# revision 31
# speedup vs baseline: 1.3116x; 1.1239x over previous
"""Trainium2 Bass kernel for additive (Bahdanau) attention.

Reference computation (per batch b):
    qp = queries @ Wq                    # (Tq, H)
    kp = keys @ Wk                       # (Tk, H)
    scores[q,k] = sum_h wv[h] * tanh(qp[q,h] + kp[k,h])
    attn = softmax(scores masked to k < valid_lens[b])
    out = attn @ values                  # (Tq, D)

Shapes: B=8, Tq=128, Tk=512, D=256, H=256 (fp32).

Strategy: separable harmonic expansion of tanh + key-chunk sharding.

The baseline's cost was the (q,k,h) tanh feature tensor on ScalarE
(1 elem/cycle/lane, ~58us on the critical core). This kernel removes
that tensor entirely: tanh(a+b) is approximated by an odd-harmonic sine
series  tanh(s) ~= sum_m c_m sin(m*om0*s), m in {1,3,..,13},  and each
sin(m*om0*(a+b)) factors exactly as
    sin(m*om0*a)cos(m*om0*b) + cos(m*om0*a)sin(m*om0*b),
so the whole score tensor becomes a TensorE matmul with contraction
(m, sin/cos, h) of size 2*7*H. Fit (Gaussian-weighted over the realized
s-distribution, |s|<=8.7): wrms ~1.1e-3, max err @|s|<=8.5 ~9e-3 -- below
the bf16 noise floor of the baseline.

Per chunk of 128 keys, on-core:
  - qp/kp projections on TensorE (bf16 inputs, fp32 PSUM);
  - fundamentals sin(om0*p), cos(om0*p) on ScalarE straight from PSUM
    (om0=0.28 keeps |angle| <= pi/2 for |p|<=5.6, within the Sin table's
    +-pi valid range even with the +pi/2 cos bias);
  - higher odd harmonics via the stride-2 Chebyshev/angle recurrence
    X_{m+2} = 2cos(2th) * X_m - X_{m-2} on DVE in bf16 (2x packed mode),
    both sides and sin/cos batched per instruction;
  - A-side scaled once by c_m * wv_h (precomputed, replicated constant);
  - 28 accumulating TensorE matmuls -> transposed score tile scT[k,q];
  - softmax via exp(scT + bias) with the global bound M = sum|wv|+1
    (partials combine across chunks by plain summation; Exp instructions
    for all chunks are grouped after all Sin instructions so the
    activation-table switch happens exactly once);
  - attn-partial @ [values | 1] on TensorE accumulates the denominator.
Host sums per-chunk [128, 257] partials per batch and divides.

Valid-length chunk planning as the baseline: only chunks with k <
valid_lens[b] are computed; chunks are padded to a uniform U per core.
"""

import math
import numpy as np
import ml_dtypes
from contextlib import ExitStack

import concourse.bass as bass
import concourse.tile as tile
from concourse import bacc, mybir
from concourse import bass_utils

B, Tq, Tk, D, H = 8, 128, 512, 256, 256
N_CORES = 8
KC = 128          # keys per chunk
F32 = mybir.dt.float32
BF16 = mybir.dt.bfloat16
NEG_BIG = -1.0e9

# odd-harmonic sine fit of tanh: tanh(s) ~= sum_j CM[j] sin((2j+1)*OM0*s)
OM0 = 0.34
CM = [1.2088745, 0.29297413, 0.08475813, 0.04523419]
KH = len(CM)

SIN = mybir.ActivationFunctionType.Sin
EXP = mybir.ActivationFunctionType.Exp
MULT = mybir.AluOpType.mult
ADD = mybir.AluOpType.add
SUB = mybir.AluOpType.subtract


def _bcast(ap_slice, axis_idx, count):
    """Insert a step-0 (broadcast) dim into an AP (axis_idx includes the
    partition dim at index 0)."""
    ap = list(ap_slice.ap)
    ap.insert(axis_idx, [0, count])
    return bass.AP(tensor=ap_slice.tensor, offset=ap_slice.offset, ap=ap)


def _flat(ap_slice, keep=0):
    """Merge the trailing free dims of a contiguous slice into one long
    row (DVE pays a per-row overhead, so fewer/longer rows are faster).
    `keep` leading free dims are preserved (e.g. a step-0 broadcast dim)."""
    ap = list(ap_slice.ap)
    head, tail = ap[: 1 + keep], ap[1 + keep :]
    n = 1
    for _, ct in tail:
        n *= ct
    return bass.AP(tensor=ap_slice.tensor, offset=ap_slice.offset,
                   ap=head + [[1, n]])


def _emit(nc, tc, ins, out_dram, U):
    with ExitStack() as ctx:
        const = ctx.enter_context(tc.tile_pool(name="const", bufs=1))
        chunk_in = ctx.enter_context(tc.tile_pool(name="chunk_in", bufs=2))
        feat = ctx.enter_context(tc.tile_pool(name="feat", bufs=2))
        scal = ctx.enter_context(tc.tile_pool(name="scal", bufs=2))
        pt_pool = ctx.enter_context(tc.tile_pool(name="pt", bufs=2))
        out_pool = ctx.enter_context(tc.tile_pool(name="outs", bufs=2))
        proj_ps = ctx.enter_context(tc.tile_pool(name="proj_ps", bufs=2, space="PSUM"))
        sc_ps_pool = ctx.enter_context(tc.tile_pool(name="sc_ps", bufs=1, space="PSUM"))
        av_ps_pool = ctx.enter_context(tc.tile_pool(name="av_ps", bufs=2, space="PSUM"))

        # pi/2 bias for the cos-via-sin fundamentals
        halfpi = const.tile([128, 1], F32)
        nc.vector.memset(halfpi, float(np.pi / 2))

        wk_sb = const.tile([128, 2, H], BF16)
        wq_sb = const.tile([128, 2, H], BF16)
        # wvc shipped tiny ([128, KH, half]) and replicated on-device over
        # (sc, q) by two ScalarE Copies -- saves ~650KB of DMA traffic.
        wvc_in = const.tile([128, KH, 2], BF16)
        wvc_sb = const.tile([128, KH, 2, 2, Tq], BF16)

        # ---- inputs on three DMA queues, balanced so the projection inputs
        # all land ~together: the combined all-chunk kT/qT transfers (packed
        # [128, U, 2, T] on host) ride with one weight half each; the other
        # weight halves go via the scalar queue.
        queues = [nc.sync, nc.gpsimd, nc.scalar]
        kT_sb = chunk_in.tile([128, U, 2, KC], BF16, tag="kT")
        qT_sb = chunk_in.tile([128, U, 2, Tq], BF16, tag="qT")
        nc.scalar.dma_start(out=wvc_in, in_=ins["wvc"])
        nc.sync.dma_start(out=wk_sb, in_=ins["wk"])
        nc.gpsimd.dma_start(out=wq_sb, in_=ins["wq"])
        nc.sync.dma_start(out=kT_sb, in_=ins["kT_u"])
        nc.gpsimd.dma_start(out=qT_sb, in_=ins["qT_u"])

        # Dummy activation: pulls the Sin table load off the critical path
        # (runs while the DMAs above are in flight).
        warm_sb = const.tile([1, 1], F32)
        nc.vector.memset(warm_sb, 0.0)
        nc.scalar.activation(warm_sb, warm_sb, SIN)
        # PE pipeline warmup.
        warm_w = const.tile([1, 2], BF16)
        nc.vector.memset(warm_w, 0.0)
        wp = av_ps_pool.tile([1, 1], F32, tag="avo")
        nc.tensor.matmul(wp, warm_w[:, 0:1], warm_w[:, 1:2], start=True, stop=True)

        # ---- phase A: all-chunk projections + two big fundamentals ----
        # X layout: [128p(h), KH, sc(sin=0,cos=1), U, side(q=0,k=1), half, T]
        X = feat.tile([128, KH, 2, U, 2, 2, Tq], BF16, tag="X")
        # one PSUM region [U, side, half, T] matching X's fundamental slice
        pk_ps = proj_ps.tile([128, U, 2, 2, Tq], F32, tag="pk")
        for u in range(U):
            for half in range(2):
                hs = slice(half * 128, (half + 1) * 128)
                for dc in range(2):
                    nc.tensor.matmul(
                        pk_ps[:, u, 1, half, :], wk_sb[:, dc, hs],
                        kT_sb[:, u, dc, :], start=(dc == 0), stop=(dc == 1))
            for half in range(2):
                hs = slice(half * 128, (half + 1) * 128)
                for dc in range(2):
                    nc.tensor.matmul(
                        pk_ps[:, u, 0, half, :], wq_sb[:, dc, hs],
                        qT_sb[:, u, dc, :], start=(dc == 0), stop=(dc == 1))
        # fundamentals: sin first (it alone gates the ladder's t2/dpm chain)
        nc.scalar.activation(X[:, 0, 0], pk_ps, SIN, scale=OM0)
        nc.scalar.activation(X[:, 0, 1], pk_ps, SIN, bias=halfpi, scale=OM0)
        # Replicate wvc over (sc, q) on ScalarE -- after the fundamentals so
        # it never delays them; only needed once the ladder reaches wvc01.
        # in: [p][2,KH][1,2][0,Tq] (m, half packed, q bcast); out per sc.
        wvc_flat = bass.AP(tensor=wvc_in.tensor, offset=wvc_in.offset,
                           ap=[list(wvc_in.ap[0]), [2, KH], [1, 2], [0, Tq]])
        for sc in range(2):
            o = wvc_sb[:, :, sc]
            o_ap = bass.AP(tensor=o.tensor, offset=o.offset,
                           ap=[list(o.ap[0]), [2 * 2 * Tq, KH], [Tq, 2], [1, Tq]])
            nc.scalar.activation(o_ap, wvc_flat,
                                 mybir.ActivationFunctionType.Copy)

        # ---- odd-harmonic ladder on DVE, all chunks batched in one op set --
        s1 = X[:, 0, 0]   # [128, U, side, half, T]
        t2 = scal.tile([128, U, 2, 2, Tq], BF16, tag="t2")
        nc.vector.tensor_tensor(out=t2, in0=s1, in1=s1, op=MULT)
        # dpm[0] = d+1 = 3-4s1^2 (pairs sin), dpm[1] = d-1 (pairs cos)
        dpm = scal.tile([128, 2, U, 2, 2, Tq], BF16, tag="dpm")
        dd = scal.tile([128, U, 2, 2, Tq], BF16, tag="dd")    # d = 2-4s1^2
        nc.vector.tensor_scalar(out=dpm[:, 0], in0=t2, scalar1=-4.0,
                                scalar2=3.0, op0=MULT, op1=ADD)
        nc.vector.tensor_scalar(out=dpm[:, 1], in0=t2, scalar1=-4.0,
                                scalar2=1.0, op0=MULT, op1=ADD)
        nc.vector.tensor_scalar(out=dd, in0=t2, scalar1=-4.0,
                                scalar2=2.0, op0=MULT, op1=ADD)
        Ap = feat.tile([128, KH, 2, U, 2, Tq], BF16, tag="Ap")

        def emit_wvc(mr):
            # A-side scale of levels [mr] by c_m * wv_h (bcast over U)
            nc.vector.tensor_tensor(
                out=Ap[:, mr], in0=X[:, mr, :, :, 0],
                in1=_bcast(wvc_sb[:, mr], 3, U), op=MULT)

        # m=3: X[1] = X[0] * dpm   (sc-paired multipliers)
        nc.vector.tensor_tensor(out=X[:, 1], in0=X[:, 0], in1=dpm, op=MULT)
        emit_wvc(slice(0, 2))
        # m>=5: X[lv] = d*X[lv-1] - X[lv-2]   (d bcast over sc); per-level
        # wvc lets the PE start that level's score matmuls immediately.
        for lv in range(2, KH):
            P = scal.tile([128, 2, U, 2, 2, Tq], BF16, tag="P")
            nc.vector.tensor_tensor(out=P, in0=X[:, lv - 1],
                                    in1=_bcast(dd, 1, 2), op=MULT)
            nc.vector.tensor_tensor(out=X[:, lv], in0=P, in1=X[:, lv - 2],
                                    op=SUB)
            emit_wvc(slice(lv, lv + 1))

        # ---- scores: scT[k,q] += B_chunk^T A_chunk over (m, sc, half) ----
        scts = [sc_ps_pool.tile([128, Tq], F32, tag=f"scT{u}", name=f"scT{u}")
                for u in range(U)]
        for m in range(KH):
            for pi, (scb, sca) in enumerate(((1, 0), (0, 1))):
                for u in range(U):
                    for half in range(2):
                        nc.tensor.matmul(
                            scts[u], X[:, m, scb, u, 1, half, :],
                            Ap[:, m, sca, u, half, :],
                            start=(m == 0 and pi == 0 and half == 0),
                            stop=(m == KH - 1 and pi == 1 and half == 1))

        # ---- phase B: all Exp instructions grouped (one table switch) ----
        for u in range(U):
            b_eng = queues[u % 2]  # sync / gpsimd (keep scalar free for ACT)
            v_sb = chunk_in.tile([128, D + 1], BF16, tag="v")
            b_eng.dma_start(out=v_sb, in_=ins["v_u"][u])
            mb_sb = chunk_in.tile([128, 1], F32, tag="mb")
            b_eng.dma_start(out=mb_sb, in_=ins["mb_u"][u])
            pT_sb = pt_pool.tile([128, Tq], BF16, tag="pT")
            nc.scalar.activation(pT_sb, scts[u], EXP, bias=mb_sb[:, 0:1], scale=1.0)
            av_ps = av_ps_pool.tile([Tq, D + 1], F32, tag="avo")
            nc.tensor.matmul(av_ps, pT_sb, v_sb, start=True, stop=True)
            out_sb = out_pool.tile([Tq, D + 1], F32, tag="out")
            nc.vector.tensor_copy(out_sb, av_ps)
            nc.sync.dma_start(out=out_dram[u], in_=out_sb)


def _build(U):
    nc = bacc.Bacc(
        "TRN2",
        target_bir_lowering=False,
        debug=False,
        enable_asserts=False,
        num_devices=N_CORES,
    )
    ins = {
        "wq": nc.dram_tensor("wq", [128, 2, H], BF16, kind="ExternalInput").ap(),
        "wk": nc.dram_tensor("wk", [128, 2, H], BF16, kind="ExternalInput").ap(),
        "wvc": nc.dram_tensor("wvc", [128, KH, 2], BF16, kind="ExternalInput").ap(),
        "qT_u": nc.dram_tensor("qT_u", [128, U, 2, Tq], BF16, kind="ExternalInput").ap(),
        "kT_u": nc.dram_tensor("kT_u", [128, U, 2, KC], BF16, kind="ExternalInput").ap(),
        "v_u": nc.dram_tensor("v_u", [U, KC, D + 1], BF16, kind="ExternalInput").ap(),
        "mb_u": nc.dram_tensor("mb_u", [U, KC, 1], F32, kind="ExternalInput").ap(),
    }
    out_dram = nc.dram_tensor("out_u", [U, Tq, D + 1], F32, kind="ExternalOutput").ap()
    with tile.TileContext(nc) as tc:
        _emit(nc, tc, ins, out_dram, U)
    nc.compile()
    return nc


_NC_CACHE = {}


def _get_nc(U):
    if U not in _NC_CACHE:
        _NC_CACHE[U] = _build(U)
    return _NC_CACHE[U]


def _plan_chunks(valid_lens):
    chunks = []
    for b in range(B):
        n = int(valid_lens[b])
        for kc in range(math.ceil(max(n, 0) / KC)):
            chunks.append((b, kc))
    U = max(1, math.ceil(len(chunks) / N_CORES))
    chunks += [None] * (N_CORES * U - len(chunks))
    return chunks, U


def run(queries, keys, values, valid_lens, Wq, Wk, wv, trace=False):
    """Run the SPMD kernel; returns (output, BassKernelResults)."""
    queries = np.asarray(queries, dtype=np.float32)
    keys = np.asarray(keys, dtype=np.float32)
    values = np.asarray(values, dtype=np.float32)
    valid_lens = np.asarray(valid_lens)

    def pmajor(a):
        # [d, ...] -> [p, c, ...] with d = c*128 + p, contiguous
        return np.ascontiguousarray(
            a.reshape(2, 128, *a.shape[1:]).swapaxes(0, 1)
        )

    Wq_p = pmajor(np.asarray(Wq, dtype=np.float32).astype(ml_dtypes.bfloat16))
    Wk_p = pmajor(np.asarray(Wk, dtype=np.float32).astype(ml_dtypes.bfloat16))
    wv_bf = np.asarray(wv, dtype=np.float32).astype(ml_dtypes.bfloat16)
    # scores are bounded by ~sum|wv|; M makes exp(s-M) overflow-safe without
    # a row max, so partial softmax sums combine by addition.
    M = float(np.abs(wv_bf.astype(np.float32)).sum()) + 1.0

    # wvc[p, m, half] = CM[m] * wv[half*128 + p] (device replicates over sc, q)
    wv_ph = wv_bf.astype(np.float32).reshape(2, 128).T        # [128p, 2half]
    wvc = np.ascontiguousarray(
        np.asarray(CM, np.float32)[None, :, None] * wv_ph[:, None, :]
    ).astype(ml_dtypes.bfloat16)

    chunks, U = _plan_chunks(valid_lens)
    nc = _get_nc(U)

    # [B, D, T] transposed inputs, packed partition-major per batch
    qT = np.stack([pmajor(queries[b].T.astype(ml_dtypes.bfloat16)) for b in range(B)])
    kT = np.stack([pmajor(keys[b].T.astype(ml_dtypes.bfloat16)) for b in range(B)])
    ones = np.ones((KC, 1), dtype=np.float32)
    arange = np.arange(KC)

    in_maps = []
    for c in range(N_CORES):
        qT_u = np.zeros((128, U, 2, Tq), ml_dtypes.bfloat16)
        kT_u = np.zeros((128, U, 2, KC), ml_dtypes.bfloat16)
        v_u = np.zeros((U, KC, D + 1), ml_dtypes.bfloat16)
        mb_u = np.full((U, KC, 1), NEG_BIG - M, np.float32)
        for u in range(U):
            ch = chunks[c * U + u]
            if ch is None:
                continue
            b, kc = ch
            k0 = kc * KC
            qT_u[:, u] = qT[b]
            kT_u[:, u] = kT[b][:, :, k0 : k0 + KC]
            v_u[u] = np.concatenate([values[b][k0 : k0 + KC], ones], axis=1).astype(
                ml_dtypes.bfloat16
            )
            mb_u[u, :, 0] = (
                np.where(k0 + arange < int(valid_lens[b]), 0.0, NEG_BIG) - M
            ).astype(np.float32)
        in_maps.append(
            {
                "wq": Wq_p,
                "wk": Wk_p,
                "wvc": wvc,
                "qT_u": qT_u,
                "kT_u": kT_u,
                "v_u": v_u,
                "mb_u": mb_u,
            }
        )

    res = bass_utils.run_bass_kernel_spmd(
        nc, in_maps, core_ids=list(range(N_CORES)), trace=trace
    )

    acc = np.zeros((B, Tq, D + 1), np.float64)
    for c in range(N_CORES):
        part = res.results[c]["out_u"]  # [U, Tq, D+1]
        for u in range(U):
            ch = chunks[c * U + u]
            if ch is None:
                continue
            acc[ch[0]] += part[u]
    out = np.zeros((B, Tq, D), np.float32)
    for b in range(B):
        if int(valid_lens[b]) > 0:
            out[b] = (acc[b, :, :D] / acc[b, :, D : D + 1]).astype(np.float32)
    return out, res


def kernel(queries, keys, values, valid_lens, Wq, Wk, wv):
    out, _ = run(queries, keys, values, valid_lens, Wq, Wk, wv, trace=False)
    return out
